# revision 54
# baseline (speedup 1.0000x reference)
"""Trainium2 Bass kernel for the CNN_PHMM_VAE loss (profile-HMM forward + KLD).

Strategy: pure data parallel over batch (512 -> 8 cores x 64). Each core runs
the 256-column HMM forward recurrence in linear space over a [64 batch
partitions, 129 motif positions] state, entirely on the Vector engine in
bf16, with FIVE fused instructions per column. Fusion uses stride-0
outer-repeat access patterns (a [BS, 129] operand read twice as [BS, 2, 129])
so one instruction produces two logical outputs:

  opA: U  = [FI x2]   * [ci|gr]   -> [u1-half | w-half]     (tt, 258)
  opB: TF = [FM_p x2] + U         -> [t-half  | FI-half]    (tt, 258)
  t2 : t2 = G_p + t                                         (tt, 128)
  opW: W  = [t2pad x2] * V        -> [d1-half | FM-half]    (tt, 258)
  scan: G[1:] = scan(q2, d1)                                (128)

State units (host precompute, fp64): FM~ = FM/PM as in the log-space
reference reparametrization; FI^ absorbs the M2I coefficient; the delete
chain runs in G = cdfull * FD^ units so its u2-multiply folds into the scan
coefficients q2 and the d1 table V (V[k] = cdfull[k]*em[k-1] interleaved
with the em half). Emissions are gathered/exp'd on the host and used RAW
(log em is zero-mean, so the state random-walks within the rescale margin;
no per-column normalization). Numeric range: per-batch anchor C_b plus a
runtime rescale every RS columns; z is measured from the accum slots of
the opA/t2 stt variants at event columns (covering both the M- and
I-lineage scales), the reciprocal r rides the scalar slots of the
post-event column's stt variants, and the host applies all log
corrections and the final mean. KLD is computed on-device.
"""
import sys

sys.path.insert(0, "/opt/trn_rl_repo")

import os

import numpy as np
import ml_dtypes

bf16 = ml_dtypes.bfloat16

B, L, K, E = 512, 256, 128, 16
L = int(os.environ.get("PHMM_L", L))  # internal: small-L perf probes only
REPEAT = int(os.environ.get("PHMM_REPEAT", 1))  # internal: perf probes only
NCORES = 8
BS = B // NCORES
Kp1 = K + 1
RS = int(os.environ.get("PHMM_RS", 4))
NEV = max(L // RS - 1, 1)   # rescale events at l = RS, 2RS, ..., L-RS
NCHUNK = 8
CCOLS = L // NCHUNK
W2 = 2 * Kp1   # 258

M2M, M2I, M2D, I2M, I2I, D2M, D2D = 0, 1, 2, 3, 4, 5, 6

_cache = {}


def _build_program():
    import concourse.bacc as bacc
    import concourse.tile as tile
    from concourse import mybir

    f32 = mybir.dt.float32
    b16 = mybir.dt.bfloat16
    Alu = mybir.AluOpType

    nc = bacc.Bacc("TRN2", target_bir_lowering=False, debug=False)

    v_dram = nc.declare_dram_parameter("vt", [BS, L, 2, Kp1], b16, isOutput=False)
    cig_d = nc.declare_dram_parameter("cig", [BS, 2, Kp1], b16, isOutput=False)
    q2_d = nc.declare_dram_parameter("q2", [BS, Kp1], b16, isOutput=False)
    fm0_d = nc.declare_dram_parameter("fm0", [BS, Kp1], b16, isOutput=False)
    g0_d = nc.declare_dram_parameter("g0", [BS, Kp1], b16, isOutput=False)
    icb_d = nc.declare_dram_parameter("icb", [BS, 1], f32, isOutput=False)
    al_d = nc.declare_dram_parameter("al", [BS, 3], f32, isOutput=False)
    mus_d = nc.declare_dram_parameter("mus", [BS, E], f32, isOutput=False)
    lv_d = nc.declare_dram_parameter("lv", [BS, E], f32, isOutput=False)
    v_d = nc.declare_dram_parameter("outv", [BS, 1], f32, isOutput=True)
    z_d = nc.declare_dram_parameter("outz", [BS, NEV], f32, isOutput=True)
    kld_d = nc.declare_dram_parameter("outk", [BS, 1], f32, isOutput=True)

    with tile.TileContext(nc) as tc:
        with tc.tile_pool(name="p", bufs=1) as pool:
            def T(shape, tag, dt=b16):
                return pool.tile(shape, dt, tag=tag, name=tag)

            vt = [T([BS, CCOLS, 2, Kp1], f"vt{j}") for j in range(NCHUNK)]
            cig = T([BS, 2, Kp1], "cig")
            q2 = T([BS, Kp1], "q2")
            icb = T([BS, 1], "icb", f32)
            al = T([BS, 3], "al", f32)
            mus_t = T([BS, E], "mus", f32); lv_t = T([BS, E], "lv", f32)

            w_t = T([BS, 2, Kp1], "w")        # [d1pad | FM]
            tf_t = T([BS, 2, Kp1], "tf")      # [tpad | FIpre]
            u_t = T([BS, 2, Kp1], "u")        # [u1 | w]
            g_t = T([BS, Kp1], "g")
            t2p = T([BS, Kp1], "t2p")         # t2pad: [0, t2[0..127]]
            zm = T([BS, NEV], "zm", f32); z2 = T([BS, NEV], "z2", f32)
            zbuf = T([BS, NEV], "zbuf", f32)
            r_t = T([BS, 1], "r", f32)
            w0_t = T([BS, 1], "w0", f32); w1_t = T([BS, 1], "w1", f32)
            v_t = T([BS, 1], "v", f32)
            m2_t = T([BS, E], "m2", f32); s1_t = T([BS, E], "s1", f32)
            ee_t = T([BS, E], "ee", f32); s2_t = T([BS, E], "s2", f32)
            red_t = T([BS, 1], "red", f32); kld_t = T([BS, 1], "kld", f32)

            # small tables first so column 1 never waits on the big stream
            nc.sync.dma_start(cig[:], cig_d[:])
            nc.sync.dma_start(q2[:], q2_d[:])
            nc.sync.dma_start(icb[:], icb_d[:]); nc.sync.dma_start(al[:], al_d[:])
            nc.sync.dma_start(g_t[:], g0_d[:])
            nc.sync.dma_start(w_t[:, 1, :], fm0_d[:])
            nc.sync.dma_start(mus_t[:], mus_d[:]); nc.sync.dma_start(lv_t[:], lv_d[:])
            for j in range(NCHUNK):
                nc.sync.dma_start(
                    vt[j][:], v_dram[:, j * CCOLS:(j + 1) * CCOLS])

            nc.vector.memset(tf_t[:], 0.0)
            nc.vector.memset(t2p[:], 0.0)
            nc.vector.memset(w_t[:, 0, :], 0.0)

            # KLD = -0.5 * sum(1 + lv - mus^2 - exp(lv)); DVE + ACT, one-time.
            nc.vector.tensor_tensor(m2_t[:], mus_t[:], mus_t[:], Alu.mult)
            nc.vector.tensor_tensor(s1_t[:], lv_t[:], m2_t[:], Alu.subtract)
            nc.scalar.activation(ee_t[:], lv_t[:], mybir.ActivationFunctionType.Exp)
            nc.vector.scalar_tensor_tensor(
                out=s2_t[:], in0=s1_t[:], scalar=1.0, in1=ee_t[:],
                op0=Alu.mult, op1=Alu.subtract, accum_out=red_t[:])
            nc.scalar.activation(
                kld_t[:], red_t[:], mybir.ActivationFunctionType.Copy,
                bias=-0.5 * E, scale=-0.5)
            nc.sync.dma_start(kld_d[:], kld_t[:])

            def rep2(ap):
                # [BS, n] -> [BS, 2, n] stride-0 outer repeat
                return ap.unsqueeze(1).broadcast_to([BS, 2, ap.shape[-1]])

            post_event = False
            ev = 0
            for l0 in range(1, REPEAT * L + 1):
                l = (l0 - 1) % L + 1
                if l == 1 and l0 > 1:
                    ev = 0
                    post_event = False
                    nc.sync.dma_start(g_t[:], g0_d[:])
                    nc.sync.dma_start(w_t[:, 1, :], fm0_d[:])
                    nc.vector.memset(tf_t[:], 0.0)
                j, c = (l - 1) // CCOLS, (l - 1) % CCOLS
                vsl = vt[j][:, c]
                is_event = (l % RS == 0 and l < L)
                scB = r_t[:, 0:1] if post_event else 1.0

                # opA: U = [FI x2] * [ci|gr]  (raw emissions: no rho; the
                # post-event rescale r rides the scalar slot so the whole
                # I-lineage scales consistently with FM/G)
                if post_event:
                    nc.vector.scalar_tensor_tensor(
                        out=u_t[:], in0=rep2(tf_t[:, 1, :]), scalar=scB,
                        in1=cig[:], op0=Alu.mult, op1=Alu.mult)
                else:
                    nc.vector.tensor_tensor(
                        u_t[:], rep2(tf_t[:, 1, :]), cig[:], Alu.mult)
                # opB: TF = ([FM_p x2] * sc5) + U  (tt when sc5 == 1)
                # (+z2 accum at events: sum(t + FI) tracks the M-lineage via
                # t = u1 + FM, the I-lineage via FI, and the G-lineage with a
                # one-column lag through t2 -> FM -> t)
                if post_event:
                    nc.vector.scalar_tensor_tensor(
                        out=tf_t[:], in0=rep2(w_t[:, 1, :]), scalar=scB,
                        in1=u_t[:], op0=Alu.mult, op1=Alu.add)
                elif is_event:
                    nc.vector.scalar_tensor_tensor(
                        out=tf_t[:], in0=rep2(w_t[:, 1, :]), scalar=1.0,
                        in1=u_t[:], op0=Alu.mult, op1=Alu.add,
                        accum_out=z2[:, ev:ev + 1])
                else:
                    nc.vector.tensor_tensor(
                        tf_t[:], rep2(w_t[:, 1, :]), u_t[:], Alu.add)
                # t2: t2pad[1:] = (G_p[0:K] * sc5) + t
                if post_event:
                    nc.vector.scalar_tensor_tensor(
                        out=t2p[:, 1:Kp1], in0=g_t[:, 0:K], scalar=scB,
                        in1=tf_t[:, 0, 0:K], op0=Alu.mult, op1=Alu.add)
                else:
                    nc.vector.tensor_tensor(
                        t2p[:, 1:Kp1], g_t[:, 0:K], tf_t[:, 0, 0:K], Alu.add)
                # opW: W = [t2pad x2] * V
                nc.vector.tensor_tensor(
                    w_t[:], rep2(t2p[:]), vsl, Alu.mult)
                # scan: G[1:] = scan(q2, d1)
                nc.vector.tensor_tensor_scan(
                    out=g_t[:, 1:Kp1], data0=q2[:, 1:Kp1],
                    data1=w_t[:, 0, 0:K],
                    initial=0.0, op0=Alu.mult, op1=Alu.add)

                post_event = False
                if is_event:
                    # zbuf = z2*icb; r = 1/zbuf
                    nc.vector.tensor_scalar(
                        zbuf[:, ev:ev + 1], z2[:, ev:ev + 1], icb[:, 0:1],
                        None, Alu.mult)
                    nc.vector.reciprocal(r_t[:], zbuf[:, ev:ev + 1])
                    post_event = True
                    ev += 1

            # readout v = aM*FM[K] + aI'*FIpre[K] + aD'*G[K]
            # (al[1] carries rho_L, al[2] = sM2M[K]; host folds the rest)
            nc.vector.scalar_tensor_tensor(
                out=w0_t[:], in0=w_t[:, 1, K:Kp1], scalar=al[:, 0:1],
                in1=w_t[:, 1, K:Kp1], op0=Alu.mult, op1=Alu.bypass)
            nc.vector.scalar_tensor_tensor(
                out=w1_t[:], in0=tf_t[:, 1, K:Kp1], scalar=al[:, 1:2],
                in1=w0_t[:], op0=Alu.mult, op1=Alu.add)
            nc.vector.scalar_tensor_tensor(
                out=v_t[:], in0=g_t[:, K:Kp1], scalar=al[:, 2:3],
                in1=w1_t[:], op0=Alu.mult, op1=Alu.add)
            nc.sync.dma_start(v_d[:], v_t[:])
            nc.sync.dma_start(z_d[:], zbuf[:])

    nc.compile()
    return nc


def _precompute(batch_input, a, e_m):
    """Host precompute in fp64. Returns device tables + host corrections."""
    a = a.astype(np.float64)
    sM2M = np.exp(a[:, :, M2M]); sI2M = np.exp(a[:, :, I2M])
    sD2M = np.exp(a[:, :, D2M]); sM2I4 = 0.25 * np.exp(a[:, :, M2I])
    sI2I4 = 0.25 * np.exp(a[:, :, I2I]); sM2D = np.exp(a[:, :, M2D])
    Bn = a.shape[0]

    Dhat = np.ones((Bn, Kp1))
    Dhat[:, 1:] = sM2D[:, :-1] / sM2M[:, :-1]
    cdfull = sD2M * Dhat / sM2M      # k = 0..K
    cI = (sI2M * sM2I4 / sM2M)[:, :K]
    grow = sI2I4                      # k = 0..K
    lq = np.zeros((Bn, Kp1))
    lq[:, 1:] = (a[:, :-1, D2D] + np.log(Dhat[:, :-1]) - np.log(Dhat[:, 1:])
                 - a[:, :-1, M2M])
    q = np.exp(lq); q[:, 0] = 0.0
    lcdf = np.log(cdfull)
    q2 = np.zeros((Bn, Kp1))
    q2[:, 1:] = np.exp(lq[:, 1:] + lcdf[:, 1:] - lcdf[:, :-1])

    # per-batch anchor from the max drawup of the q2-prefix walk
    pref = np.cumsum(lq, axis=1)
    runmin = np.minimum.accumulate(pref, axis=1)
    Qspread = np.max(pref - runmin, axis=1)
    lcD = lcdf[:, :K].max(axis=1)
    headD = Qspread + np.maximum(lcD, 0.0)
    margin = 25.0 if RS <= 4 else 33.0
    logCb = np.minimum(45.0, 88.0 - margin - headD)
    Cb = np.exp(logCb)

    logPMK = a[:, :K, M2M].sum(axis=1)

    bi = np.arange(Bn)[:, None, None]
    ki = np.arange(K)[None, None, :]
    # raw emissions: log em is zero-mean, so the state random-walks within
    # the rescale margin; no per-column normalization needed.
    EM = np.exp(e_m.astype(np.float64)[bi, ki, batch_input[:, :, None]])  # (B,L,K)

    # al: [aM, aI, aD/cdfull[K] = sM2M[K]]
    alphas = np.stack([sM2M[:, K],
                       sI2M[:, K] * sM2I4[:, K],
                       sM2M[:, K]], axis=1)

    fm0 = np.zeros((Bn, Kp1))
    fm0[:, 0] = Cb

    # fd0 chain from fm0 (host fp64): fd0[k] = q[k]*fd0[k-1] + fm0[k-1]
    fd0 = np.zeros((Bn, Kp1))
    for k in range(1, Kp1):
        fd0[:, k] = q[:, k] * fd0[:, k - 1] + fm0[:, k - 1]
    g0 = cdfull * fd0

    # V table per column: [d1 half (129) | em half (129)]
    #   V[:, l, j]      = cdfull[j+1]*EM[l, j-1]  for j=1..127 (else 0)
    #   V[:, l, 129+s]  = EM[l, s-1]              for s=1..128 (V[129] = 0)
    V = np.zeros((Bn, L, W2))
    V[:, :, 1:K] = cdfull[:, None, 2:Kp1] * EM[:, :, :K - 1]
    V[:, :, Kp1 + 1:] = EM

    # cig: [ci_pad (129) | gr (129)]
    cig = np.zeros((Bn, W2))
    cig[:, :K] = cI
    cig[:, Kp1:] = grow

    tables = dict(
        vt=V.reshape(Bn, L, 2, Kp1).astype(bf16),
        cig=cig.reshape(Bn, 2, Kp1).astype(bf16), q2=q2.astype(bf16),
        fm0=fm0.astype(bf16), g0=g0.astype(bf16),
        icb=(1.0 / Cb)[:, None].astype(np.float32),
        al=alphas.astype(np.float32),
    )
    corr = dict(logCb=logCb, logPMK=logPMK)
    return tables, corr


def _get_exec():
    """Build program + a cached jitted shard_map executor (one compile)."""
    if "exec" in _cache:
        return _cache["exec"]
    import jax
    from jax.sharding import Mesh, PartitionSpec
    from jax.experimental.shard_map import shard_map
    from concourse import mybir
    from concourse.bass2jax import (
        install_neuronx_cc_hook, _bass_exec_p, partition_id_tensor)

    nc = _build_program()
    install_neuronx_cc_hook()

    pname = nc.partition_id_tensor.name if nc.partition_id_tensor else None
    in_names, out_names, out_avals = [], [], []
    for alloc in nc.m.functions[0].allocations:
        if not isinstance(alloc, mybir.MemoryLocationSet):
            continue
        name = alloc.memorylocations[0].name
        if alloc.kind == "ExternalInput":
            if name != pname:
                in_names.append(name)
        elif alloc.kind == "ExternalOutput":
            shape = tuple(alloc.tensor_shape)
            dtype = mybir.dt.np(alloc.dtype)
            out_names.append(name)
            out_avals.append(jax.core.ShapedArray(shape, dtype))
    n_params = len(in_names)
    all_names = in_names + out_names
    if pname is not None:
        all_names = all_names + [pname]
    donate = tuple(range(n_params, n_params + len(out_names)))

    def _body(*args):
        operands = list(args)
        if pname is not None:
            operands.append(partition_id_tensor())
        outs = _bass_exec_p.bind(
            *operands, out_avals=tuple(out_avals), in_names=tuple(all_names),
            out_names=tuple(out_names), lowering_input_output_aliases=(),
            sim_require_finite=True, sim_require_nnan=True, nc=nc)
        return tuple(outs)

    devices = jax.devices()[:NCORES]
    mesh = Mesh(np.asarray(devices), ("core",))
    in_specs = (PartitionSpec("core"),) * (n_params + len(out_names))
    out_specs = (PartitionSpec("core"),) * len(out_names)
    sharded = jax.jit(
        shard_map(_body, mesh=mesh, in_specs=in_specs, out_specs=out_specs,
                  check_rep=False),
        donate_argnums=donate, keep_unused=True)
    _cache["exec"] = (sharded, in_names, out_names, out_avals, n_params)
    return _cache["exec"]


def _run_device(tables_full):
    """tables_full: dict name -> full [B, ...] array. Returns dict of outputs
    concatenated over cores as [B, ...]."""
    sharded, in_names, out_names, out_avals, n_params = _get_exec()
    ins = [np.ascontiguousarray(tables_full[n]) for n in in_names]
    zeros = [np.zeros((NCORES * a.shape[0], *a.shape[1:]), a.dtype)
             for a in out_avals]
    outs = sharded(*ins, *zeros)
    return {n: np.asarray(o) for n, o in zip(out_names, outs)}


def kernel(batch_input, transition_probs, emission_probs, mus, logvars):
    batch_input = np.asarray(batch_input).astype(np.int64)
    a = np.asarray(transition_probs, dtype=np.float32)
    e_m = np.asarray(emission_probs, dtype=np.float32)
    mus = np.asarray(mus, dtype=np.float32)
    logvars = np.asarray(logvars, dtype=np.float32)

    tables, corr = _precompute(batch_input, a, e_m)
    tables["mus"] = mus
    tables["lv"] = logvars

    out = _run_device(tables)
    v = out["outv"][:, 0]
    z = out["outz"]
    kld = out["outk"][:, 0]

    v64 = np.maximum(v.astype(np.float64), 1e-300)
    z64 = np.maximum(z.astype(np.float64), 1e-300)
    logCb = corr["logCb"]
    # outz holds (ZM+Z2)*icb per event; log z - logCb = log outz.
    nll = -(np.log(v64) - logCb + np.log(z64).sum(axis=1)
            + corr["logPMK"])
    loss = nll.mean() + kld.astype(np.float64).mean()
    return np.float32(loss)
